# revision 1
# baseline (speedup 1.0000x reference)
"""DeepSpeed-style self-attention block (RMSNorm + QKV + RoPE + causal attention
+ output projection) on 8 Trainium2 NeuronCores.

Sharding: tensor-parallel over heads (16 heads -> 2 per core). Each core computes
its 2 heads' attention over the full sequence and a partial output projection over
its 256-dim slice of the context; the 8 partial outputs are summed on the host
(the TP all-reduce equivalent, done at gather time).

Layouts (per core, all device matmuls in float32r: full PE rate, ~1e-4 rms rounding):
  xT      [2048, 4096]   x transposed (d-major) so projections need no on-device transpose
  wqT/wkT/wvT [2048, 256] weight slices pre-transposed on host, RMSNorm weight folded in
  woT     [256, 2048]    o-proj slice pre-transposed
  cosT/sinT [128, 4096]  RoPE tables (sin sign-folded); scaled on device by the
                         per-token RMS factor s_n so RoPE eviction applies the norm free
  masks   [4, 128, 512]  causal -10000 masks for the 4 diagonal k-tile offsets

Per 512-token chunk: one pass over 16 d-tiles accumulates qT/kT (per head) and
v (transposed layout) in PSUM plus the x^2 column sums ([1,512] via ones-matmul);
s = rsqrt(mean+eps) feeds the table scaling, the v eviction (via PE transpose to
natural [token, dv] layout + tensor_scalar), and RoPE eviction of q/k. Attention
runs in scores-transposed layout [k,q] (softmax denominator = ones-matmul over the
exp tiles, accumulated alongside probs@v in PSUM), so no max-subtraction and no
transposes are needed (scores are bounded, exp(-10000 + s) underflows to exactly 0).
"""
import sys
sys.path.insert(0, '/opt/trn_rl_repo')

import math
import numpy as np
from contextlib import ExitStack

import concourse.bass as bass
from concourse import bacc
import concourse.mybir as mybir
import concourse.tile as tile
from concourse import bass_utils
from concourse.masks import make_identity

# ---- problem constants (hardcoded per contest contract) ----
B, S, H, HEADS, D = 2, 2048, 2048, 16, 128
NT = B * S                    # 4096 tokens
NCORES = 8
HPC = HEADS // NCORES         # 2 heads per core
OC = HPC * D                  # 256 output dims per core
P = 128
CH = 512                      # token chunk
NCH = NT // CH                # 8 chunks
KT = H // P                   # 16 d-tiles
CPB = S // CH                 # 4 chunks per batch
SCALE = 1.0 / math.sqrt(D)
RMS_EPS = 1e-6
ROPE_BASE = 10000.0
MASK_VAL = -10000.0

F32 = mybir.dt.float32
F32R = mybir.dt.float32r
EXP = mybir.ActivationFunctionType.Exp
SQRT = mybir.ActivationFunctionType.Sqrt


def build_module():
    nc = bacc.Bacc("TRN2", target_bir_lowering=False, debug=False, num_devices=NCORES)

    xT = nc.dram_tensor("xT", [H, NT], F32R, kind="ExternalInput").ap()
    wqT = nc.dram_tensor("wqT", [H, OC], F32R, kind="ExternalInput").ap()
    wkT = nc.dram_tensor("wkT", [H, OC], F32R, kind="ExternalInput").ap()
    wvT = nc.dram_tensor("wvT", [H, OC], F32R, kind="ExternalInput").ap()
    woT = nc.dram_tensor("woT", [OC, H], F32R, kind="ExternalInput").ap()
    cosT = nc.dram_tensor("cosT", [D, NT], F32, kind="ExternalInput").ap()
    sinT = nc.dram_tensor("sinT", [D, NT], F32, kind="ExternalInput").ap()
    masks = nc.dram_tensor("masks", [CPB, P, CH], mybir.dt.bfloat16, kind="ExternalInput").ap()
    ones_col = nc.dram_tensor("ones_col", [P, 1], F32R, kind="ExternalInput").ap()
    out_p = nc.dram_tensor("out_p", [NT, H], F32, kind="ExternalOutput").ap()

    with tile.TileContext(nc) as tc, ExitStack() as ctx:
        const = ctx.enter_context(tc.tile_pool(name="const", bufs=1))
        wpool = ctx.enter_context(tc.tile_pool(name="wpool", bufs=1))
        kvpool = ctx.enter_context(tc.tile_pool(name="kvpool", bufs=1))
        xt_pool = ctx.enter_context(tc.tile_pool(name="xtp", bufs=4))
        sq_pool = ctx.enter_context(tc.tile_pool(name="sqp", bufs=2))
        trig = ctx.enter_context(tc.tile_pool(name="trig", bufs=2))
        rope_t = ctx.enter_context(tc.tile_pool(name="ropet", bufs=2))
        q_pool = ctx.enter_context(tc.tile_pool(name="qp", bufs=3))
        vt_pool = ctx.enter_context(tc.tile_pool(name="vtp", bufs=2))
        ex_pool = ctx.enter_context(tc.tile_pool(name="exp", bufs=3))
        ctx_pool = ctx.enter_context(tc.tile_pool(name="ctxp", bufs=2))
        o_pool = ctx.enter_context(tc.tile_pool(name="op", bufs=2))
        small1 = ctx.enter_context(tc.tile_pool(name="small1", bufs=1))
        small = ctx.enter_context(tc.tile_pool(name="small", bufs=2))
        bc_pool = ctx.enter_context(tc.tile_pool(name="bcp", bufs=2))
        ps = ctx.enter_context(tc.tile_pool(name="ps", bufs=8, space="PSUM"))

        # ---- resident constants ----
        ones_sb = const.tile([P, 1], F32R)
        nc.sync.dma_start(out=ones_sb, in_=ones_col)
        eps_sb = const.tile([1, 1], F32)
        nc.vector.memset(eps_sb, RMS_EPS)
        ident = const.tile([P, P], F32)
        make_identity(nc, ident)
        mask_sb = const.tile([P, CPB, CH], mybir.dt.bfloat16)
        nc.sync.dma_start(out=mask_sb, in_=masks.rearrange("j p q -> p j q"))
        wq_sb = wpool.tile([P, KT, OC], F32R)
        nc.sync.dma_start(out=wq_sb, in_=wqT.rearrange("(t p) o -> p t o", p=P))
        wk_sb = wpool.tile([P, KT, OC], F32R)
        nc.sync.dma_start(out=wk_sb, in_=wkT.rearrange("(t p) o -> p t o", p=P))
        wv_sb = wpool.tile([P, KT, OC], F32R)
        nc.sync.dma_start(out=wv_sb, in_=wvT.rearrange("(t p) o -> p t o", p=P))
        wo_sb = wpool.tile([P, HPC, H], F32R)
        nc.sync.dma_start(out=wo_sb, in_=woT.rearrange("(t p) o -> p t o", p=P))

        # per-chunk K/V caches, resident for the whole kernel
        k_chunks = [kvpool.tile([P, HPC, CH], F32R, name=f"kc{i}") for i in range(NCH)]
        v_chunks = [kvpool.tile([P, CPB, OC], F32R, name=f"vc{i}") for i in range(NCH)]

        pending = []

        def emit_oproj(m0, csb):
            for j in range(CPB):
                for oc in range(H // CH):
                    op_ = ps.tile([P, CH], F32, tag="ps", name=f"o{m0}_{j}_{oc}")
                    for h in range(HPC):
                        nc.tensor.matmul(op_, csb[:, h, j * P:(j + 1) * P],
                                         wo_sb[:, h, oc * CH:(oc + 1) * CH],
                                         start=(h == 0), stop=(h == HPC - 1))
                    ot = o_pool.tile([P, CH], F32, tag="ot")
                    nc.scalar.copy(ot, op_)
                    nc.sync.dma_start(
                        out=out_p[m0 + j * P:m0 + (j + 1) * P, oc * CH:(oc + 1) * CH],
                        in_=ot)

        for ich in range(NCH):
            b, li = ich // CPB, ich % CPB
            n0 = ich * CH

            # ---- pass A: projections + x^2 stats over 16 d-tiles ----
            qp = [ps.tile([P, CH], F32, tag="ps", name=f"qp{ich}_{h}") for h in range(HPC)]
            kp = [ps.tile([P, CH], F32, tag="ps", name=f"kp{ich}_{h}") for h in range(HPC)]
            vp = [ps.tile([P, CH], F32, tag="ps", name=f"vp{ich}_{h}") for h in range(HPC)]
            ssp = ps.tile([1, CH], F32, tag="ps", name=f"ss{ich}")
            for dt in range(KT):
                xt = xt_pool.tile([P, CH], F32R, tag="xt")
                nc.sync.dma_start(out=xt, in_=xT[dt * P:(dt + 1) * P, n0:n0 + CH])
                xq = sq_pool.tile([P, CH], F32R, tag="xq")
                nc.vector.tensor_mul(xq, xt, xt)
                st, sp = (dt == 0), (dt == KT - 1)
                for h in range(HPC):
                    nc.tensor.matmul(qp[h], wq_sb[:, dt, h * P:(h + 1) * P], xt, start=st, stop=sp)
                    nc.tensor.matmul(kp[h], wk_sb[:, dt, h * P:(h + 1) * P], xt, start=st, stop=sp)
                    nc.tensor.matmul(vp[h], wv_sb[:, dt, h * P:(h + 1) * P], xt, start=st, stop=sp)
                nc.tensor.matmul(ssp, ones_sb, xq, start=st, stop=sp)

            while pending:
                emit_oproj(*pending.pop(0))

            # ---- RMS scale: s = 1/sqrt(mean(x^2)+eps), row and column forms ----
            s_sqrt = small1.tile([1, CH], F32, tag="ssq")
            nc.scalar.activation(s_sqrt, ssp, SQRT, bias=eps_sb, scale=1.0 / H)
            s_row = small.tile([1, CH], F32, tag="srow")
            nc.vector.reciprocal(s_row, s_sqrt)
            s_bc = bc_pool.tile([P, CH], F32, tag="bc")
            nc.gpsimd.partition_broadcast(s_bc, s_row)

            cosS = trig.tile([P, CH], F32, tag="cosS")
            nc.sync.dma_start(out=cosS, in_=cosT[:, n0:n0 + CH])
            sinS = trig.tile([P, CH], F32, tag="sinS")
            nc.sync.dma_start(out=sinS, in_=sinT[:, n0:n0 + CH])
            nc.vector.tensor_mul(cosS, cosS, s_bc)
            nc.vector.tensor_mul(sinS, sinS, s_bc)

            # ---- RoPE + scale eviction of q, k (psum [d,512] -> f32r sbuf) ----
            HD = D // 2
            q_sb = []
            for h in range(HPC):
                for (psum_t, dst) in ((qp[h], None), (kp[h], k_chunks[ich][:, h, :])):
                    t1 = rope_t.tile([P, CH], F32, tag="t1")
                    nc.vector.tensor_mul(t1, psum_t, cosS)
                    t2 = rope_t.tile([P, CH], F32, tag="t2")
                    nc.vector.tensor_mul(t2[0:HD, :], psum_t[HD:P, :], sinS[0:HD, :])
                    nc.vector.tensor_mul(t2[HD:P, :], psum_t[0:HD, :], sinS[HD:P, :])
                    if dst is None:
                        dst = q_pool.tile([P, CH], F32R, tag="q")
                        q_sb.append(dst)
                    nc.vector.tensor_add(dst, t1, t2)

            # ---- V eviction: psum [o,512] -> transpose -> scaled natural [n, o] ----
            for h in range(HPC):
                vts = vt_pool.tile([P, CH], F32, tag="vts")
                nc.vector.tensor_mul(vts, vp[h], s_bc)
                for j in range(CPB):
                    tpp = ps.tile([P, P], F32, tag="ps", name=f"tp{ich}_{h}_{j}")
                    nc.tensor.transpose(tpp, vts[:, j * P:(j + 1) * P], ident)
                    nc.scalar.copy(v_chunks[ich][:, j, h * P:(h + 1) * P], tpp)

            # ---- attention for this q-chunk, per head ----
            nkt = CPB * (li + 1)
            ctx_sb = ctx_pool.tile([P, HPC, CH], F32R, tag="ctx")
            for h in range(HPC):
                ctxp = ps.tile([P, CH], F32, tag="ps", name=f"cx{ich}_{h}")
                denp = ps.tile([1, CH], F32, tag="ps", name=f"dn{ich}_{h}")
                for kt in range(nkt):
                    ck = b * CPB + kt // CPB
                    j = kt % CPB
                    sp_ = ps.tile([P, CH], F32, tag="ps", name=f"s{ich}_{h}_{kt}")
                    nc.tensor.matmul(sp_, k_chunks[ck][:, h, j * P:(j + 1) * P], q_sb[h],
                                     start=True, stop=True)
                    dj = kt - CPB * li
                    if dj >= 0:
                        nc.vector.tensor_add(sp_, sp_, mask_sb[:, dj, :])
                    ex = ex_pool.tile([P, CH], F32R, tag="ex")
                    nc.scalar.activation(ex, sp_, EXP, scale=SCALE)
                    st, last = (kt == 0), (kt == nkt - 1)
                    nc.tensor.matmul(ctxp, v_chunks[ck][:, j, h * P:(h + 1) * P], ex,
                                     start=st, stop=last)
                    nc.tensor.matmul(denp, ones_sb, ex, start=st, stop=last)
                den_s = small.tile([1, CH], F32, tag="dens")
                nc.scalar.copy(den_s, denp)
                rec = small.tile([1, CH], F32, tag="rec")
                nc.vector.reciprocal(rec, den_s)
                rbc = bc_pool.tile([P, CH], F32, tag="bc")
                nc.gpsimd.partition_broadcast(rbc, rec)
                nc.vector.tensor_mul(ctx_sb[:, h, :], ctxp, rbc)

            # ---- partial o-proj deferred into the next chunk's stats bubble ----
            pending.append((n0, ctx_sb))

        while pending:
            emit_oproj(*pending.pop(0))

    nc.compile()
    return nc


def prep_inputs(x, norm_w, wq, wk, wv, wo, position_ids):
    """Host-side sharding/layout prep. Returns per-core input maps."""
    x = np.asarray(x, dtype=np.float32)
    norm_w = np.asarray(norm_w, dtype=np.float32)
    wq = np.asarray(wq, dtype=np.float32)
    wk = np.asarray(wk, dtype=np.float32)
    wv = np.asarray(wv, dtype=np.float32)
    wo = np.asarray(wo, dtype=np.float32)
    pos = np.asarray(position_ids)

    xT = np.ascontiguousarray(x.reshape(NT, H).T)

    # RoPE tables from position_ids, sign-folded sin
    inv_freq = 1.0 / (ROPE_BASE ** (np.arange(0, D, 2, dtype=np.float32) / D))
    t = pos.reshape(NT).astype(np.float32)
    freqs = np.einsum("n,f->nf", t, inv_freq)
    emb = np.concatenate([freqs, freqs], axis=1)          # [NT, D]
    cos = np.cos(emb).astype(np.float32)
    sin = np.sin(emb).astype(np.float32)
    sinF = sin.copy()
    sinF[:, :D // 2] *= -1.0
    cosT = np.ascontiguousarray(cos.T)
    sinT = np.ascontiguousarray(sinF.T)

    # diagonal-block causal masks: mask[j][kk, qq] = 0 if qq >= j*128+kk else -1e4
    qq = np.arange(CH)[None, None, :]
    kk = np.arange(P)[None, :, None]
    jj = np.arange(CPB)[:, None, None]
    import ml_dtypes
    masks = np.where(qq >= jj * P + kk, 0.0, MASK_VAL).astype(ml_dtypes.bfloat16)

    ones_col = np.ones((P, 1), dtype=np.float32)

    wq_f = wq * norm_w[None, :]
    wk_f = wk * norm_w[None, :]
    wv_f = wv * norm_w[None, :]

    in_maps = []
    for c in range(NCORES):
        sl = slice(c * OC, (c + 1) * OC)
        in_maps.append({
            "xT": xT,
            "wqT": np.ascontiguousarray(wq_f[sl].T),
            "wkT": np.ascontiguousarray(wk_f[sl].T),
            "wvT": np.ascontiguousarray(wv_f[sl].T),
            "woT": np.ascontiguousarray(wo[:, sl].T),
            "cosT": cosT,
            "sinT": sinT,
            "masks": masks,
            "ones_col": ones_col,
        })
    return in_maps


_NC_CACHE = None


def _get_module():
    global _NC_CACHE
    if _NC_CACHE is None:
        _NC_CACHE = build_module()
    return _NC_CACHE


def kernel(x, norm_w, wq, wk, wv, wo, position_ids):
    nc = _get_module()
    in_maps = prep_inputs(x, norm_w, wq, wk, wv, wo, position_ids)
    res = bass_utils.run_bass_kernel_spmd(nc, in_maps, core_ids=list(range(NCORES)))
    acc = np.zeros((NT, H), dtype=np.float64)
    for c in range(NCORES):
        acc += res.results[c]["out_p"].astype(np.float64)
    return acc.astype(np.float32).reshape(B, S, H)



# revision 4
# speedup vs baseline: 1.2898x; 1.2898x over previous
"""DeepSpeed-style self-attention block (RMSNorm + QKV + RoPE + causal attention
+ output projection) on 8 Trainium2 NeuronCores.

Sharding: TP4 x DP2. Cores 0-3 take batch 0, cores 4-7 batch 1; within a batch
group core g owns heads 4g..4g+3 (wq/wk/wv output slice, wo input slice). Each
core computes its batch's full sequence for its 4 heads plus a partial output
projection [2048, 2048]; the host sums the 4 partials per batch (TP all-reduce
at gather time).

Data path is bf16 end-to-end on the PE (both matmul operands), f32 PSUM
accumulation, f32 scalar math (RMS scale, softmax denominators), f32 output
partials. RMS rsqrt is computed as exp(-0.5*ln(m)) so the whole kernel uses a
single activation table (natural_log_exp) - no ACT table swaps.

Per 512-token chunk:
  stats:  xq = x*x (DVE stt), ones-matmul accumulates ssp [1,512] in the aux
          PSUM bank; s = exp(-0.5 ln(ssp/H + eps)); s broadcast and folded into
          the per-chunk cos/sin tables (so RoPE eviction applies the norm free)
          and into V via one stt per head.
  qkv:    per head, q/k accumulate into one 2-bank PSUM tile ([d, tok] layout),
          ACT-evicts to bf16, RoPE on DVE (4x mode) writes q tile + K cache.
          V accumulates per head in [dv, tok], stt-scales by s, PE-transposes
          to natural [tok, dv] bf16, ACT-evicts into the V cache.
  attn:   scores in [k, q] layout (no transposes), exp with no max-subtraction
          (scores bounded; masked lanes underflow to exactly 0), denominator =
          ones-matmul accumulated in the aux bank alongside probs@v.
  oproj:  [tok, out] = ctx^T slices @ wo, 4-head PSUM accumulation, evicted and
          DMA'd per [128, 512] tile; overlaps the next chunk's projections.
"""
import sys
sys.path.insert(0, '/opt/trn_rl_repo')

import math
import numpy as np
from contextlib import ExitStack

import concourse.bass as bass
from concourse import bacc
import concourse.mybir as mybir
import concourse.tile as tile
from concourse import bass_utils
from concourse.masks import make_identity

# ---- problem constants (hardcoded per contest contract) ----
B, S, H, HEADS, D = 2, 2048, 2048, 16, 128
NCORES = 8
TPG = 4                       # tensor-parallel group size
HPC = HEADS // TPG            # 4 heads per core
OC = HPC * D                  # 512 proj output dims per core
T = S                         # tokens per core (one batch each)
P = 128
CH = 512                      # token chunk
NCH = T // CH                 # 4 chunks
KT = H // P                   # 16 d-tiles
HD = D // 2
SCALE = 1.0 / math.sqrt(D)
RMS_EPS = 1e-6
ROPE_BASE = 10000.0
MASK_VAL = -10000.0

F32 = mybir.dt.float32
F32R = mybir.dt.float32r
BF16 = mybir.dt.bfloat16
EXP = mybir.ActivationFunctionType.Exp
LN = mybir.ActivationFunctionType.Ln
MULT = mybir.AluOpType.mult
ADD = mybir.AluOpType.add


def build_module():
    nc = bacc.Bacc("TRN2", target_bir_lowering=False, debug=False, num_devices=NCORES)

    xT = nc.dram_tensor("xT", [H, T], BF16, kind="ExternalInput").ap()
    wqT = nc.dram_tensor("wqT", [H, OC], BF16, kind="ExternalInput").ap()
    wkT = nc.dram_tensor("wkT", [H, OC], BF16, kind="ExternalInput").ap()
    wvT = nc.dram_tensor("wvT", [H, OC], BF16, kind="ExternalInput").ap()
    woT = nc.dram_tensor("woT", [OC, H], BF16, kind="ExternalInput").ap()
    cosT = nc.dram_tensor("cosT", [D, T], F32, kind="ExternalInput").ap()
    sinT = nc.dram_tensor("sinT", [D, T], F32, kind="ExternalInput").ap()
    masks = nc.dram_tensor("masks", [NCH, P, CH], BF16, kind="ExternalInput").ap()
    ones_col = nc.dram_tensor("ones_col", [P, 1], BF16, kind="ExternalInput").ap()
    out_p = nc.dram_tensor("out_p", [T, H], F32, kind="ExternalOutput").ap()

    stt = None  # set below per-engine

    with tile.TileContext(nc) as tc, ExitStack() as ctx:
        const = ctx.enter_context(tc.tile_pool(name="const", bufs=1))
        wpool = ctx.enter_context(tc.tile_pool(name="wpool", bufs=1))
        kvpool = ctx.enter_context(tc.tile_pool(name="kv", bufs=1))
        xt_pool = ctx.enter_context(tc.tile_pool(name="xtp", bufs=20))
        sq_pool = ctx.enter_context(tc.tile_pool(name="sqp", bufs=3))
        trigc = ctx.enter_context(tc.tile_pool(name="trigc", bufs=2))
        qke_pool = ctx.enter_context(tc.tile_pool(name="qke", bufs=3))
        rt_pool = ctx.enter_context(tc.tile_pool(name="rt", bufs=4))
        q_pool = ctx.enter_context(tc.tile_pool(name="qp", bufs=HPC + 2))
        vts_pool = ctx.enter_context(tc.tile_pool(name="vts", bufs=2))
        ex_pool = ctx.enter_context(tc.tile_pool(name="ex", bufs=6))
        cs_pool = ctx.enter_context(tc.tile_pool(name="cs", bufs=HPC + 2))
        oev_pool = ctx.enter_context(tc.tile_pool(name="oev", bufs=4))
        small = ctx.enter_context(tc.tile_pool(name="small", bufs=3))
        bc_pool = ctx.enter_context(tc.tile_pool(name="bcp", bufs=2))
        # PSUM: qk 2 + vp 1 + spop 2 + ctx 1 + aux 2 = 8 banks
        qk_ps = ctx.enter_context(tc.tile_pool(name="qkps", bufs=1, space="PSUM"))
        spop_ps = ctx.enter_context(tc.tile_pool(name="spop", bufs=2, space="PSUM"))
        vp_ps = ctx.enter_context(tc.tile_pool(name="vpps", bufs=1, space="PSUM"))
        ctx_ps = ctx.enter_context(tc.tile_pool(name="cxps", bufs=1, space="PSUM"))
        den_ps = ctx.enter_context(tc.tile_pool(name="denps", bufs=2, space="PSUM"))

        vstt = nc.vector.scalar_tensor_tensor

        # ---- resident constants ----
        ones_sb = const.tile([P, 1], BF16)
        nc.sync.dma_start(out=ones_sb, in_=ones_col)
        eps_sb = const.tile([1, 1], F32)
        nc.vector.memset(eps_sb, RMS_EPS)
        ident = const.tile([P, P], BF16)
        make_identity(nc, ident)

        # chunk-0 x tiles first so PE can start before weights finish
        xts0 = []
        for dt in range(KT):
            xt_ = xt_pool.tile([P, CH], BF16, tag="xt")
            nc.sync.dma_start(out=xt_, in_=xT[dt * P:(dt + 1) * P, 0:CH])
            xts0.append(xt_)

        # weights, split into 4 DMAs each so the first matmuls start early
        wq_sb = wpool.tile([P, KT, OC], BF16)
        wk_sb = wpool.tile([P, KT, OC], BF16)
        wv_sb = wpool.tile([P, KT, OC], BF16)
        for w_sb, wdram in ((wq_sb, wqT), (wk_sb, wkT), (wv_sb, wvT)):
            wr = wdram.rearrange("(t p) o -> p t o", p=P)
            for g in range(4):
                nc.sync.dma_start(out=w_sb[:, g * 4:(g + 1) * 4, :],
                                  in_=wr[:, g * 4:(g + 1) * 4, :])
        cosF = const.tile([P, T], F32)
        nc.sync.dma_start(out=cosF, in_=cosT)
        sinF = const.tile([P, T], F32)
        nc.sync.dma_start(out=sinF, in_=sinT)
        mask_sb = const.tile([P, NCH, CH], BF16)
        nc.sync.dma_start(out=mask_sb, in_=masks.rearrange("j p q -> p j q"))
        wo_sb = wpool.tile([P, HPC, H], BF16)
        wor = woT.rearrange("(t p) o -> p t o", p=P)
        for g in range(HPC):
            nc.sync.dma_start(out=wo_sb[:, g, :], in_=wor[:, g, :])

        # K cache [d, head, chunk, tok]; V cache natural [tok, chunk, j, head, dv]
        k_sb = kvpool.tile([P, HPC, NCH, CH], BF16)
        v_sb = kvpool.tile([P, NCH, 4, HPC, P], BF16)

        for c in range(NCH):
            n0 = c * CH

            # ---- x tiles for this chunk (chunk 0 preloaded) ----
            if c == 0:
                xts = xts0
            else:
                xts = []
                for dt in range(KT):
                    xt_ = xt_pool.tile([P, CH], BF16, tag="xt")
                    nc.sync.dma_start(out=xt_,
                                      in_=xT[dt * P:(dt + 1) * P, n0:n0 + CH])
                    xts.append(xt_)

            ssp = den_ps.tile([1, CH], F32, tag="den", name=f"ssp{c}")

            # ---- RMS stats: ssp = sum_d x^2, then s = exp(-0.5 ln(m+eps)) ----
            for dt in range(KT):
                xq = sq_pool.tile([P, CH], BF16, tag="xq")
                nc.vector.tensor_mul(xq, xts[dt], xts[dt])
                nc.tensor.matmul(ssp, ones_sb, xq,
                                 start=(dt == 0), stop=(dt == KT - 1))
            s_ln = small.tile([1, CH], F32, tag="sc")
            nc.scalar.activation(s_ln, ssp, LN, bias=eps_sb, scale=1.0 / H)
            s_row = small.tile([1, CH], F32, tag="sc")
            nc.scalar.activation(s_row, s_ln, EXP, scale=-0.5)
            s_bc = bc_pool.tile([P, CH], F32, tag="sbc")
            nc.gpsimd.partition_broadcast(s_bc, s_row)
            cosS = trigc.tile([P, CH], BF16, tag="cosS")
            nc.vector.tensor_mul(cosS, cosF[:, n0:n0 + CH], s_bc)
            sinS = trigc.tile([P, CH], BF16, tag="sinS")
            nc.vector.tensor_mul(sinS, sinF[:, n0:n0 + CH], s_bc)

            # ---- QKV + RoPE per head ----
            q_tiles = []
            for h in range(HPC):
                qk = qk_ps.tile([P, 2, CH], F32, tag="qk", name=f"qk{c}_{h}")
                for dt in range(KT):
                    st, sp_ = (dt == 0), (dt == KT - 1)
                    nc.tensor.matmul(qk[:, 0, :], wq_sb[:, dt, h * P:(h + 1) * P],
                                     xts[dt], start=st, stop=sp_)
                    nc.tensor.matmul(qk[:, 1, :], wk_sb[:, dt, h * P:(h + 1) * P],
                                     xts[dt], start=st, stop=sp_)
                vp = vp_ps.tile([P, CH], F32, tag="vp", name=f"vp{c}_{h}")
                for dt in range(KT):
                    nc.tensor.matmul(vp, wv_sb[:, dt, h * P:(h + 1) * P], xts[dt],
                                     start=(dt == 0), stop=(dt == KT - 1))

                qke = qke_pool.tile([P, 2, CH], BF16, tag="qke")
                nc.scalar.copy(qke, qk)
                qt = q_pool.tile([P, CH], BF16, tag="q", name=f"q{c}_{h}")
                q_tiles.append(qt)
                for idx, dst in ((0, qt), (1, k_sb[:, h, c, :])):
                    t1 = rt_pool.tile([P, CH], BF16, tag="t1")
                    nc.vector.tensor_mul(t1, qke[:, idx, :], cosS)
                    t2 = rt_pool.tile([P, CH], BF16, tag="t2")
                    # half-swapped operand must come from PSUM (the SBUF-only
                    # path trips the verifier's same-start-partition check)
                    nc.vector.tensor_mul(t2[0:HD, :], qk[HD:P, idx, :],
                                         sinS[0:HD, :])
                    nc.vector.tensor_mul(t2[HD:P, :], qk[0:HD, idx, :],
                                         sinS[HD:P, :])
                    nc.vector.tensor_add(dst, t1, t2)

                vts = vts_pool.tile([P, CH], BF16, tag="vts")
                nc.vector.tensor_mul(vts, vp, s_bc)
                tpp = spop_ps.tile([P, CH], BF16, tag="spop", name=f"tp{c}_{h}")
                for j in range(4):
                    nc.tensor.transpose(tpp[:, j * P:(j + 1) * P],
                                        vts[:, j * P:(j + 1) * P], ident)
                nc.scalar.copy(
                    v_sb[:, c, :, h, :],
                    tpp[:, :].rearrange("p (j q) -> p j q", j=4))

            # ---- attention for this q-chunk ----
            cs_tiles = []
            nkt = 4 * (c + 1)
            for h in range(HPC):
                cxp = ctx_ps.tile([P, CH], F32, tag="cxp", name=f"cx{c}_{h}")
                den = den_ps.tile([1, CH], F32, tag="den", name=f"den{c}_{h}")
                for kt in range(nkt):
                    ck, j = divmod(kt, 4)
                    sp = spop_ps.tile([P, CH], F32, tag="spop", name=f"sp{c}_{h}_{kt}")
                    nc.tensor.matmul(sp, k_sb[:, h, ck, j * P:(j + 1) * P],
                                     q_tiles[h], start=True, stop=True)
                    dj = kt - 4 * c
                    if dj >= 0:
                        nc.vector.tensor_add(sp, sp, mask_sb[:, dj, :])
                    ex = ex_pool.tile([P, CH], BF16, tag="ex")
                    nc.scalar.activation(ex, sp, EXP, scale=SCALE)
                    st, last = (kt == 0), (kt == nkt - 1)
                    nc.tensor.matmul(cxp, v_sb[:, ck, j, h, :], ex,
                                     start=st, stop=last)
                    nc.tensor.matmul(den, ones_sb, ex,
                                     start=st, stop=last)
                sden = small.tile([1, CH], F32, tag="sc")
                nc.vector.reciprocal(sden, den)
                rbc = bc_pool.tile([P, CH], F32, tag="rbc")
                nc.gpsimd.partition_broadcast(rbc, sden)
                cst = cs_pool.tile([P, CH], BF16, tag="cs", name=f"cs{c}_{h}")
                nc.vector.tensor_mul(cst, cxp, rbc)
                cs_tiles.append(cst)

            # ---- partial o-proj; overlaps the next chunk's projections ----
            for j in range(4):
                for oc_ in range(4):
                    op = spop_ps.tile([P, CH], F32, tag="spop", name=f"op{c}_{j}_{oc_}")
                    for h in range(HPC):
                        nc.tensor.matmul(op, cs_tiles[h][:, j * P:(j + 1) * P],
                                         wo_sb[:, h, oc_ * CH:(oc_ + 1) * CH],
                                         start=(h == 0), stop=(h == HPC - 1))
                    ot = oev_pool.tile([P, CH], F32, tag="ot")
                    if (j + oc_) % 2 == 0:
                        nc.scalar.copy(ot, op)
                    else:
                        nc.vector.tensor_copy(out=ot, in_=op)
                    nc.sync.dma_start(
                        out=out_p[n0 + j * P:n0 + (j + 1) * P,
                                  oc_ * CH:(oc_ + 1) * CH],
                        in_=ot)

    nc.compile()
    return nc


def prep_inputs(x, norm_w, wq, wk, wv, wo, position_ids):
    """Host-side sharding/layout prep. Returns per-core input maps."""
    import ml_dtypes
    bf16 = ml_dtypes.bfloat16
    x = np.asarray(x, dtype=np.float32)
    norm_w = np.asarray(norm_w, dtype=np.float32)
    wq = np.asarray(wq, dtype=np.float32)
    wk = np.asarray(wk, dtype=np.float32)
    wv = np.asarray(wv, dtype=np.float32)
    wo = np.asarray(wo, dtype=np.float32)
    pos = np.asarray(position_ids)

    # RoPE tables per batch from position_ids, sign-folded sin
    inv_freq = 1.0 / (ROPE_BASE ** (np.arange(0, D, 2, dtype=np.float32) / D))
    wq_f = wq * norm_w[None, :]
    wk_f = wk * norm_w[None, :]
    wv_f = wv * norm_w[None, :]

    qq = np.arange(CH)[None, None, :]
    kk = np.arange(P)[None, :, None]
    jj = np.arange(NCH)[:, None, None]
    masks = np.where(qq >= jj * P + kk, 0.0, MASK_VAL).astype(bf16)
    ones_col_ = np.ones((P, 1), dtype=bf16)

    in_maps = []
    for core in range(NCORES):
        b, g = core // TPG, core % TPG
        t = pos[b].astype(np.float32)
        freqs = np.einsum("n,f->nf", t, inv_freq)
        emb = np.concatenate([freqs, freqs], axis=1)      # [T, D]
        cos = np.cos(emb).astype(np.float32)
        sin = np.sin(emb).astype(np.float32)
        sinF = sin.copy()
        sinF[:, :HD] *= -1.0
        sl = slice(g * OC, (g + 1) * OC)
        in_maps.append({
            "xT": np.ascontiguousarray(x[b].T).astype(bf16),
            "wqT": np.ascontiguousarray(wq_f[sl].T).astype(bf16),
            "wkT": np.ascontiguousarray(wk_f[sl].T).astype(bf16),
            "wvT": np.ascontiguousarray(wv_f[sl].T).astype(bf16),
            "woT": np.ascontiguousarray(wo[:, sl].T).astype(bf16),
            "cosT": np.ascontiguousarray(cos.T),
            "sinT": np.ascontiguousarray(sinF.T),
            "masks": masks,
            "ones_col": ones_col_,
        })
    return in_maps


_NC_CACHE = None


def _get_module():
    global _NC_CACHE
    if _NC_CACHE is None:
        _NC_CACHE = build_module()
    return _NC_CACHE


def kernel(x, norm_w, wq, wk, wv, wo, position_ids):
    nc = _get_module()
    in_maps = prep_inputs(x, norm_w, wq, wk, wv, wo, position_ids)
    res = bass_utils.run_bass_kernel_spmd(nc, in_maps, core_ids=list(range(NCORES)))
    out = np.zeros((B, T, H), dtype=np.float64)
    for core in range(NCORES):
        out[core // TPG] += res.results[core]["out_p"].astype(np.float64)
    return out.astype(np.float32)


# revision 5
# speedup vs baseline: 1.3309x; 1.0318x over previous
"""DeepSpeed-style self-attention block (RMSNorm + QKV + RoPE + causal attention
+ output projection) on 8 Trainium2 NeuronCores.

Sharding: TP4 x DP2. Cores 0-3 take batch 0, cores 4-7 batch 1; within a batch
group core g owns heads 4g..4g+3 (wq/wk/wv output slice, wo input slice). Each
core computes its batch's full sequence for its 4 heads plus a partial output
projection [2048, 2048]; the host sums the 4 partials per batch (TP all-reduce
at gather time).

Data path is bf16 end-to-end on the PE (both matmul operands), f32 PSUM
accumulation, f32 scalar math (RMS scale, softmax denominators), f32 output
partials. RMS rsqrt is computed as exp(-0.5*ln(m)) so the whole kernel uses a
single activation table (natural_log_exp) - no ACT table swaps.

Per 512-token chunk:
  stats:  xq = x*x (DVE stt), ones-matmul accumulates ssp [1,512] in the aux
          PSUM bank; s = exp(-0.5 ln(ssp/H + eps)); s broadcast and folded into
          the per-chunk cos/sin tables (so RoPE eviction applies the norm free)
          and into V via one stt per head.
  qkv:    per head, q/k accumulate into one 2-bank PSUM tile ([d, tok] layout),
          ACT-evicts to bf16, RoPE on DVE (4x mode) writes q tile + K cache.
          V accumulates per head in [dv, tok], stt-scales by s, PE-transposes
          to natural [tok, dv] bf16, ACT-evicts into the V cache.
  attn:   scores in [k, q] layout (no transposes), exp with no max-subtraction
          (scores bounded; masked lanes underflow to exactly 0), denominator =
          ones-matmul accumulated in the aux bank alongside probs@v.
  oproj:  [tok, out] = ctx^T slices @ wo, 4-head PSUM accumulation, evicted and
          DMA'd per [128, 512] tile; overlaps the next chunk's projections.
"""
import sys
sys.path.insert(0, '/opt/trn_rl_repo')

import math
import numpy as np
from contextlib import ExitStack

import concourse.bass as bass
from concourse import bacc
import concourse.mybir as mybir
import concourse.tile as tile
from concourse import bass_utils
from concourse.masks import make_identity

# ---- problem constants (hardcoded per contest contract) ----
B, S, H, HEADS, D = 2, 2048, 2048, 16, 128
NCORES = 8
TPG = 4                       # tensor-parallel group size
HPC = HEADS // TPG            # 4 heads per core
OC = HPC * D                  # 512 proj output dims per core
T = S                         # tokens per core (one batch each)
P = 128
CH = 512                      # token chunk
NCH = T // CH                 # 4 chunks
KT = H // P                   # 16 d-tiles
HD = D // 2
SCALE = 1.0 / math.sqrt(D)
RMS_EPS = 1e-6
ROPE_BASE = 10000.0
MASK_VAL = -10000.0

F32 = mybir.dt.float32
F32R = mybir.dt.float32r
BF16 = mybir.dt.bfloat16
EXP = mybir.ActivationFunctionType.Exp
LN = mybir.ActivationFunctionType.Ln
MULT = mybir.AluOpType.mult
ADD = mybir.AluOpType.add


def build_module():
    nc = bacc.Bacc("TRN2", target_bir_lowering=False, debug=False, num_devices=NCORES)

    xT = nc.dram_tensor("xT", [H, T], BF16, kind="ExternalInput").ap()
    wqT = nc.dram_tensor("wqT", [H, OC], BF16, kind="ExternalInput").ap()
    wkT = nc.dram_tensor("wkT", [H, OC], BF16, kind="ExternalInput").ap()
    wvT = nc.dram_tensor("wvT", [H, OC], BF16, kind="ExternalInput").ap()
    woT = nc.dram_tensor("woT", [OC, H], BF16, kind="ExternalInput").ap()
    cosT = nc.dram_tensor("cosT", [D, T], F32, kind="ExternalInput").ap()
    sinT = nc.dram_tensor("sinT", [D, T], F32, kind="ExternalInput").ap()
    masks = nc.dram_tensor("masks", [NCH, P, CH], BF16, kind="ExternalInput").ap()
    ones_col = nc.dram_tensor("ones_col", [P, 1], BF16, kind="ExternalInput").ap()
    out_p = nc.dram_tensor("out_p", [T, H], F32, kind="ExternalOutput").ap()

    stt = None  # set below per-engine

    with tile.TileContext(nc) as tc, ExitStack() as ctx:
        const = ctx.enter_context(tc.tile_pool(name="const", bufs=1))
        wpool = ctx.enter_context(tc.tile_pool(name="wpool", bufs=1))
        kvpool = ctx.enter_context(tc.tile_pool(name="kv", bufs=1))
        xt_pool = ctx.enter_context(tc.tile_pool(name="xtp", bufs=20))
        sq_pool = ctx.enter_context(tc.tile_pool(name="sqp", bufs=3))
        trigc = ctx.enter_context(tc.tile_pool(name="trigc", bufs=2))
        qke_pool = ctx.enter_context(tc.tile_pool(name="qke", bufs=3))
        rt_pool = ctx.enter_context(tc.tile_pool(name="rt", bufs=4))
        q_pool = ctx.enter_context(tc.tile_pool(name="qp", bufs=HPC + 2))
        vts_pool = ctx.enter_context(tc.tile_pool(name="vts", bufs=2))
        ex_pool = ctx.enter_context(tc.tile_pool(name="ex", bufs=6))
        cs_pool = ctx.enter_context(tc.tile_pool(name="cs", bufs=HPC + 2))
        oev_pool = ctx.enter_context(tc.tile_pool(name="oev", bufs=4))
        small = ctx.enter_context(tc.tile_pool(name="small", bufs=3))
        bc_pool = ctx.enter_context(tc.tile_pool(name="bcp", bufs=2))
        # PSUM: qk 2 + vp 1 + spop 2 + ctx 1 + aux 2 = 8 banks
        qk_ps = ctx.enter_context(tc.tile_pool(name="qkps", bufs=1, space="PSUM"))
        spop_ps = ctx.enter_context(tc.tile_pool(name="spop", bufs=2, space="PSUM"))
        vp_ps = ctx.enter_context(tc.tile_pool(name="vpps", bufs=1, space="PSUM"))
        ctx_ps = ctx.enter_context(tc.tile_pool(name="cxps", bufs=1, space="PSUM"))
        den_ps = ctx.enter_context(tc.tile_pool(name="denps", bufs=2, space="PSUM"))

        vstt = nc.vector.scalar_tensor_tensor

        # ---- resident constants ----
        ones_sb = const.tile([P, 1], BF16)
        nc.sync.dma_start(out=ones_sb, in_=ones_col)
        eps_sb = const.tile([1, 1], F32)
        nc.vector.memset(eps_sb, RMS_EPS)
        ident = const.tile([P, P], BF16)
        make_identity(nc, ident)

        # chunk-0 x tiles first so PE can start before weights finish
        xts0 = []
        for dt in range(KT):
            xt_ = xt_pool.tile([P, CH], BF16, tag="xt")
            nc.sync.dma_start(out=xt_, in_=xT[dt * P:(dt + 1) * P, 0:CH])
            xts0.append(xt_)

        # weights, split into 4 DMAs each so the first matmuls start early
        wq_sb = wpool.tile([P, KT, OC], BF16)
        wk_sb = wpool.tile([P, KT, OC], BF16)
        wv_sb = wpool.tile([P, KT, OC], BF16)
        for w_sb, wdram in ((wq_sb, wqT), (wk_sb, wkT), (wv_sb, wvT)):
            wr = wdram.rearrange("(t p) o -> p t o", p=P)
            for g in range(4):
                nc.sync.dma_start(out=w_sb[:, g * 4:(g + 1) * 4, :],
                                  in_=wr[:, g * 4:(g + 1) * 4, :])
        cosF = const.tile([P, T], F32)
        nc.sync.dma_start(out=cosF, in_=cosT)
        sinF = const.tile([P, T], F32)
        nc.sync.dma_start(out=sinF, in_=sinT)
        mask_sb = const.tile([P, NCH, CH], BF16)
        nc.sync.dma_start(out=mask_sb, in_=masks.rearrange("j p q -> p j q"))
        wo_sb = wpool.tile([P, HPC, H], BF16)
        wor = woT.rearrange("(t p) o -> p t o", p=P)
        for g in range(HPC):
            nc.sync.dma_start(out=wo_sb[:, g, :], in_=wor[:, g, :])

        # K cache [d, head, chunk, tok]; V cache natural [tok, chunk, j, head, dv]
        k_sb = kvpool.tile([P, HPC, NCH, CH], BF16)
        v_sb = kvpool.tile([P, NCH, 4, HPC, P], BF16)

        for c in range(NCH):
            n0 = c * CH

            # ---- x tiles for this chunk (chunk 0 preloaded) ----
            if c == 0:
                xts = xts0
            else:
                xts = []
                for dt in range(KT):
                    xt_ = xt_pool.tile([P, CH], BF16, tag="xt")
                    nc.sync.dma_start(out=xt_,
                                      in_=xT[dt * P:(dt + 1) * P, n0:n0 + CH])
                    xts.append(xt_)

            ssp = den_ps.tile([1, CH], F32, tag="den", name=f"ssp{c}")

            # ---- RMS stats: ssp = sum_d x^2, then s = exp(-0.5 ln(m+eps)) ----
            for dt in range(KT):
                xq = sq_pool.tile([P, CH], BF16, tag="xq")
                nc.vector.tensor_mul(xq, xts[dt], xts[dt])
                nc.tensor.matmul(ssp, ones_sb, xq,
                                 start=(dt == 0), stop=(dt == KT - 1))
            s_ln = small.tile([1, CH], F32, tag="sc")
            nc.scalar.activation(s_ln, ssp, LN, bias=eps_sb, scale=1.0 / H)
            s_row = small.tile([1, CH], F32, tag="sc")
            nc.scalar.activation(s_row, s_ln, EXP, scale=-0.5)
            s_bc = bc_pool.tile([P, CH], F32, tag="sbc")
            nc.gpsimd.partition_broadcast(s_bc, s_row)
            cosS = trigc.tile([P, CH], BF16, tag="cosS")
            nc.vector.tensor_mul(cosS, cosF[:, n0:n0 + CH], s_bc)
            sinS = trigc.tile([P, CH], BF16, tag="sinS")
            nc.vector.tensor_mul(sinS, sinF[:, n0:n0 + CH], s_bc)

            # ---- QKV + RoPE per head ----
            q_tiles = []
            for h in range(HPC):
                qk = qk_ps.tile([P, 2, CH], F32, tag="qk", name=f"qk{c}_{h}")
                for dt in range(KT):
                    st, sp_ = (dt == 0), (dt == KT - 1)
                    nc.tensor.matmul(qk[:, 0, :], wq_sb[:, dt, h * P:(h + 1) * P],
                                     xts[dt], start=st, stop=sp_)
                    nc.tensor.matmul(qk[:, 1, :], wk_sb[:, dt, h * P:(h + 1) * P],
                                     xts[dt], start=st, stop=sp_)
                vp = vp_ps.tile([P, CH], F32, tag="vp", name=f"vp{c}_{h}")
                for dt in range(KT):
                    nc.tensor.matmul(vp, wv_sb[:, dt, h * P:(h + 1) * P], xts[dt],
                                     start=(dt == 0), stop=(dt == KT - 1))

                # psum-reading rope muls first so the qk bank frees early
                # (half-swapped operand must come from PSUM: the SBUF-only
                # path trips the verifier's same-start-partition check)
                t2s = []
                for idx in (0, 1):
                    t2 = rt_pool.tile([P, CH], BF16, tag="t2")
                    nc.vector.tensor_mul(t2[0:HD, :], qk[HD:P, idx, :],
                                         sinS[0:HD, :])
                    nc.vector.tensor_mul(t2[HD:P, :], qk[0:HD, idx, :],
                                         sinS[HD:P, :])
                    t2s.append(t2)
                qke = qke_pool.tile([P, 2, CH], BF16, tag="qke")
                nc.scalar.copy(qke, qk)
                qt = q_pool.tile([P, CH], BF16, tag="q", name=f"q{c}_{h}")
                q_tiles.append(qt)
                for idx, dst in ((0, qt), (1, k_sb[:, h, c, :])):
                    t1 = rt_pool.tile([P, CH], BF16, tag="t1")
                    nc.vector.tensor_mul(t1, qke[:, idx, :], cosS)
                    nc.vector.tensor_add(dst, t1, t2s[idx])

                vts = vts_pool.tile([P, CH], BF16, tag="vts")
                nc.vector.tensor_mul(vts, vp, s_bc)
                tpp = spop_ps.tile([P, CH], BF16, tag="spop", name=f"tp{c}_{h}")
                for j in range(4):
                    nc.tensor.transpose(tpp[:, j * P:(j + 1) * P],
                                        vts[:, j * P:(j + 1) * P], ident)
                nc.scalar.copy(
                    v_sb[:, c, :, h, :],
                    tpp[:, :].rearrange("p (j q) -> p j q", j=4))

            # ---- attention for this q-chunk ----
            cs_tiles = []
            nkt = 4 * (c + 1)
            for h in range(HPC):
                cxp = ctx_ps.tile([P, CH], F32, tag="cxp", name=f"cx{c}_{h}")
                den = den_ps.tile([1, CH], F32, tag="den", name=f"den{c}_{h}")
                for kt in range(nkt):
                    ck, j = divmod(kt, 4)
                    sp = spop_ps.tile([P, CH], F32, tag="spop", name=f"sp{c}_{h}_{kt}")
                    nc.tensor.matmul(sp, k_sb[:, h, ck, j * P:(j + 1) * P],
                                     q_tiles[h], start=True, stop=True)
                    dj = kt - 4 * c
                    if dj >= 0:
                        nc.vector.tensor_add(sp, sp, mask_sb[:, dj, :])
                    ex = ex_pool.tile([P, CH], BF16, tag="ex")
                    nc.scalar.activation(ex, sp, EXP, scale=SCALE)
                    st, last = (kt == 0), (kt == nkt - 1)
                    nc.tensor.matmul(cxp, v_sb[:, ck, j, h, :], ex,
                                     start=st, stop=last)
                    nc.tensor.matmul(den, ones_sb, ex,
                                     start=st, stop=last)
                sden = small.tile([1, CH], F32, tag="sc")
                nc.vector.reciprocal(sden, den)
                rbc = bc_pool.tile([P, CH], F32, tag="rbc")
                nc.gpsimd.partition_broadcast(rbc, sden)
                cst = cs_pool.tile([P, CH], BF16, tag="cs", name=f"cs{c}_{h}")
                nc.vector.tensor_mul(cst, cxp, rbc)
                cs_tiles.append(cst)

            # ---- partial o-proj; overlaps the next chunk's projections ----
            for j in range(4):
                for oc_ in range(4):
                    op = spop_ps.tile([P, CH], F32, tag="spop", name=f"op{c}_{j}_{oc_}")
                    for h in range(HPC):
                        nc.tensor.matmul(op, cs_tiles[h][:, j * P:(j + 1) * P],
                                         wo_sb[:, h, oc_ * CH:(oc_ + 1) * CH],
                                         start=(h == 0), stop=(h == HPC - 1))
                    ot = oev_pool.tile([P, CH], F32, tag="ot")
                    if (j + oc_) % 2 == 0:
                        nc.scalar.copy(ot, op)
                    else:
                        nc.vector.tensor_copy(out=ot, in_=op)
                    nc.sync.dma_start(
                        out=out_p[n0 + j * P:n0 + (j + 1) * P,
                                  oc_ * CH:(oc_ + 1) * CH],
                        in_=ot)

    nc.compile()
    return nc


def prep_inputs(x, norm_w, wq, wk, wv, wo, position_ids):
    """Host-side sharding/layout prep. Returns per-core input maps."""
    import ml_dtypes
    bf16 = ml_dtypes.bfloat16
    x = np.asarray(x, dtype=np.float32)
    norm_w = np.asarray(norm_w, dtype=np.float32)
    wq = np.asarray(wq, dtype=np.float32)
    wk = np.asarray(wk, dtype=np.float32)
    wv = np.asarray(wv, dtype=np.float32)
    wo = np.asarray(wo, dtype=np.float32)
    pos = np.asarray(position_ids)

    # RoPE tables per batch from position_ids, sign-folded sin
    inv_freq = 1.0 / (ROPE_BASE ** (np.arange(0, D, 2, dtype=np.float32) / D))
    wq_f = wq * norm_w[None, :]
    wk_f = wk * norm_w[None, :]
    wv_f = wv * norm_w[None, :]

    qq = np.arange(CH)[None, None, :]
    kk = np.arange(P)[None, :, None]
    jj = np.arange(NCH)[:, None, None]
    masks = np.where(qq >= jj * P + kk, 0.0, MASK_VAL).astype(bf16)
    ones_col_ = np.ones((P, 1), dtype=bf16)

    in_maps = []
    for core in range(NCORES):
        b, g = core // TPG, core % TPG
        t = pos[b].astype(np.float32)
        freqs = np.einsum("n,f->nf", t, inv_freq)
        emb = np.concatenate([freqs, freqs], axis=1)      # [T, D]
        cos = np.cos(emb).astype(np.float32)
        sin = np.sin(emb).astype(np.float32)
        sinF = sin.copy()
        sinF[:, :HD] *= -1.0
        sl = slice(g * OC, (g + 1) * OC)
        in_maps.append({
            "xT": np.ascontiguousarray(x[b].T).astype(bf16),
            "wqT": np.ascontiguousarray(wq_f[sl].T).astype(bf16),
            "wkT": np.ascontiguousarray(wk_f[sl].T).astype(bf16),
            "wvT": np.ascontiguousarray(wv_f[sl].T).astype(bf16),
            "woT": np.ascontiguousarray(wo[:, sl].T).astype(bf16),
            "cosT": np.ascontiguousarray(cos.T),
            "sinT": np.ascontiguousarray(sinF.T),
            "masks": masks,
            "ones_col": ones_col_,
        })
    return in_maps


_NC_CACHE = None


def _get_module():
    global _NC_CACHE
    if _NC_CACHE is None:
        _NC_CACHE = build_module()
    return _NC_CACHE


def kernel(x, norm_w, wq, wk, wv, wo, position_ids):
    nc = _get_module()
    in_maps = prep_inputs(x, norm_w, wq, wk, wv, wo, position_ids)
    res = bass_utils.run_bass_kernel_spmd(nc, in_maps, core_ids=list(range(NCORES)))
    out = np.zeros((B, T, H), dtype=np.float64)
    for core in range(NCORES):
        out[core // TPG] += res.results[core]["out_p"].astype(np.float64)
    return out.astype(np.float32)
